# revision 1
# baseline (speedup 1.0000x reference)
"""Trainium2 Bass kernel for nn_CombinatorialClassifier.

Computation (reference):
    logits = einsum('bf,pqf->bpq', x, W) + b        # [B,P,Q]
    logp   = log_softmax(logits, axis=2)            # [B,P,Q]
    out    = take_along_axis(logp, part_idx, 2)     # [B,P,C]

Shapes: B=256, P=64, Q=128, C=1000, F=2048.

Sharding: expert-parallel over P across 8 cores (8 partitionings per
core).  Each core reads the full x and its W/b/part_idx slice and
writes its disjoint [B, 8, C] slice of the output.  No collectives.

Per-core dataflow ("orientation A" — q lives on SBUF partitions):
  - main matmul:   psum_lin[q, b] += WT_k[f,q].T @ xT_k[f,b], bias
    folded in as a K=1 accumulate matmul (bias[q] x ones[b]).
  - sumexp over q: ones[128,1].T @ exp[q,b] matmul (PE reduces over
    partitions), lse = Ln(sumexp) on ScalarE.
  - gather+logsoftmax in one PSUM group:
        psum_out[b, c] = linT[q,b].T @ OH[q,c] + lse[b].T @ (-1)[c]
    (the K=1 lse matmul also transposes lse into the partition dim).
    OH_p[q, c] = (q == part_idx[p,c]) is built per-p on DVE with an
    is_equal against a partition iota.

This walrus build only accepts ONE sync-wait command per compute/DMA
instruction, which dictates most of the structure:
  - x|W share one DMA per k-tile ("xw"); bias|ones share one DMA
    ("bo"), so each matmul joins on a single semaphore.
  - every SBUF tile is used exactly once (fresh slot) -> no
    WAR/WAW slot-release waits anywhere.
  - idx and iota for partitioning p are DMAd back-to-back so the
    SWDGE round-robin lands them on the same queue semaphore; the
    is_equal TT then joins on that one sem.
  - PSUM->SBUF result copies all run on DVE into per-(p-pair,bt)
    group tiles; each output DMA (on the ACT HWDGE) is preceded by a
    tiny ACT "observer" op that absorbs the DVE producer wait, so the
    DMA itself only carries its queue-predecessor wait.
  - bf16 for x/W (also halves their HBM traffic); the gather path is
    float32r (full-rate PE fp32).
"""

import numpy as np

B, P, Q, C, F = 256, 64, 128, 1000, 2048
NCORES = 8
PL = P // NCORES          # partitionings per core
KT = F // 128             # contraction tiles
BT = B // 128             # batch tiles for the gather matmul
C_CHUNKS = [(0, 512), (512, C - 512)]

MAIN_BF16 = True          # store/stream x,W as bf16 and matmul in bf16
GATHER_R = True           # gather/lse/sumexp matmul operands in float32r


def _build_nc():
    import concourse.bass as bass
    import concourse.tile as tile
    from concourse import mybir
    from contextlib import ExitStack

    DT = mybir.dt.float32
    HT = mybir.dt.float16
    MDT = mybir.dt.bfloat16 if MAIN_BF16 else mybir.dt.float32r
    # bf16 gather operands: 2-byte stationary loads keep the PE at full
    # rate (fp32r 4-byte weight loads measured ~2x slower per matmul)
    GDT = mybir.dt.bfloat16

    nc = bass.Bass()
    xw_d = nc.declare_dram_parameter("xw", [KT, 128, B + PL * Q], MDT,
                                     isOutput=False)
    bo_d = nc.declare_dram_parameter("bo", [1, PL * Q + B], MDT,
                                     isOutput=False)
    # idxq[q, p, :C] = part_idx[p, :] (same on every partition row) and
    # idxq[q, p, C] = q — idx and iota in ONE tensor/DMA, so the
    # is_equal TT joins on a single DMA semaphore
    idx_d = nc.declare_dram_parameter("idxq", [Q, PL, C + 1], HT,
                                      isOutput=False)
    out_d = nc.declare_dram_parameter("out", [B, PL, C], DT, isOutput=True)

    with ExitStack() as ctx:
        tc = ctx.enter_context(tile.TileContext(nc))
        singles = ctx.enter_context(tc.tile_pool(name="singles", bufs=1))
        ps_lin = ctx.enter_context(
            tc.tile_pool(name="ps_lin", bufs=2, space=bass.MemorySpace.PSUM))
        ps_sum = ctx.enter_context(
            tc.tile_pool(name="ps_sum", bufs=2, space=bass.MemorySpace.PSUM))
        ps_out = ctx.enter_context(
            tc.tile_pool(name="ps_out", bufs=4, space=bass.MemorySpace.PSUM))

        def fresh(shape, dtype, tag):
            return singles.tile(shape, dtype, tag=tag, name=tag)

        # ---- static tiles (all fresh, single-use) -------------------
        xwk = []
        for k in range(KT):
            t = fresh([128, B + PL * Q], MDT, f"xwk{k}")
            nc.sync.dma_start(out=t[:], in_=xw_d[k])
            xwk.append(t)
        bo_sb = fresh([1, PL * Q + B], MDT, "bo")
        nc.sync.dma_start(out=bo_sb[:], in_=bo_d[:])
        idx_sb = fresh([128, PL, C + 1], HT, "idxq")
        nc.sync.dma_start(out=idx_sb[:], in_=idx_d[:])

        # ACT-produced constants so the ACT-side matmuls join on ACT
        ones_col = fresh([128, 1], GDT, "ones")
        nc.scalar.activation(out=ones_col[:], in_=xwk[0][:, 0:1],
                             func=mybir.ActivationFunctionType.Copy,
                             bias=1.0, scale=0.0)
        negones_sb = fresh([1, 512], GDT, "negones")
        nc.scalar.activation(out=negones_sb[:], in_=bo_sb[0:1, 0:512],
                             func=mybir.ActivationFunctionType.Copy,
                             bias=-1.0, scale=0.0)

        obs_scratch = fresh([1, 4 * PL], DT, "obs")

        # ---- per-partitioning pipeline ------------------------------
        og_tiles = {}
        n_obs = 0
        for p in range(PL):
            psum_lin = ps_lin.tile([128, B], DT)
            # bias: K=1 matmul bias[q] x ones[b] opens the accumulation
            nc.tensor.matmul(
                psum_lin[:],
                bo_sb[:, p * Q:(p + 1) * Q],
                bo_sb[:, PL * Q:],
                start=True, stop=False)
            for k in range(KT):
                nc.tensor.matmul(
                    psum_lin[:],
                    xwk[k][:, B + p * Q:B + (p + 1) * Q],
                    xwk[k][:, :B],
                    start=False,
                    stop=(k == KT - 1),
                )

            # one-hot build for this p on DVE (single DMA sem join)
            oh_p = fresh([128, C], GDT, f"oh{p}")
            nc.vector.tensor_tensor(
                out=oh_p[:],
                in0=idx_sb[:, p, :C],
                in1=idx_sb[:, p, C:C + 1].broadcast_to((128, C)),
                op=mybir.AluOpType.is_equal,
            )

            linT = fresh([128, B], GDT, f"lin{p}")
            nc.vector.tensor_copy(linT[:], psum_lin[:])
            expT = fresh([128, B], GDT, f"exp{p}")
            nc.scalar.activation(
                out=expT[:], in_=linT[:],
                func=mybir.ActivationFunctionType.Exp)

            psum_sum = ps_sum.tile([1, B], DT)
            nc.tensor.matmul(
                psum_sum[:], ones_col[:], expT[:],
                start=True, stop=True)
            lse = fresh([1, B], GDT, f"lse{p}")
            nc.scalar.activation(
                out=lse[:], in_=psum_sum[:],
                func=mybir.ActivationFunctionType.Ln)

            pair = p // 2
            for bt in range(BT):
                bsl = slice(bt * 128, (bt + 1) * 128)
                if p % 2 == 0:
                    og_new = fresh([128, 2, C], DT, f"og{pair}_{bt}")
                    og_tiles[(pair, bt)] = og_new
                og = og_tiles[(pair, bt)]
                last_copy = None
                for (c0, cw) in C_CHUNKS:
                    psum_out = ps_out.tile([128, 512], DT)
                    nc.tensor.matmul(
                        psum_out[:, :cw],
                        linT[:, bsl],
                        oh_p[:, c0:c0 + cw],
                        start=True, stop=False)
                    nc.tensor.matmul(
                        psum_out[:, :cw],
                        lse[:, bsl],
                        negones_sb[:, :cw],
                        start=False, stop=True)
                    last_copy = nc.vector.tensor_copy(
                        og[:, p % 2, c0:c0 + cw], psum_out[:, :cw])
                if p % 2 == 1:
                    # ACT observer absorbs the DVE producer wait; the
                    # DMA then only carries its queue-predecessor wait
                    obs = nc.scalar.activation(
                        out=obs_scratch[0:1, n_obs:n_obs + 1],
                        in_=og[0:1, 1, C - 1:C],
                        func=mybir.ActivationFunctionType.Copy,
                        bias=0.0, scale=1.0)
                    n_obs += 1
                    dma = nc.scalar.dma_start(
                        out=out_d[bsl, p - 1:p + 1, :],
                        in_=og[:])
                    tile.add_dep_helper(dma.ins, obs.ins, sync=False,
                                        reason="dma after observer")

    _install_drain_split(nc)
    return nc


def _install_drain_split(nc, chunk=1):
    """The kernel-tail Drain waits on every live semaphore (~11), but
    this walrus build's CTRL_NO encoding fits only a couple of sync
    commands.  Splitting the drain into a chain of drains, each
    carrying `chunk` waits, is semantically identical (sequential SP
    sem waits).  Patch at serialization time so every consumer of
    nc.to_json_bytes() sees the legal form."""
    import copy
    import json

    orig = nc.to_json_bytes

    def patched():
        m = json.loads(orig())
        for fn in m["functions"]:
            for bb in fn["blocks"]:
                out = []
                for inst in bb["instructions"]:
                    si = inst.get("sync_info")
                    if (inst.get("opcode") == "Drain" and si
                            and si.get("on_wait")
                            and len(si["on_wait"]) > chunk):
                        waits = si["on_wait"]
                        head, keep = waits[:-chunk], waits[-chunk:]
                        for j in range(0, len(head), chunk):
                            clone = copy.deepcopy(inst)
                            clone["name"] = f"{inst['name']}-ds{j}"
                            clone["sync_info"] = {
                                "on_wait": head[j:j + chunk],
                                "on_update": [],
                            }
                            out.append(clone)
                        si["on_wait"] = keep
                    out.append(inst)
                bb["instructions"] = out
        return json.dumps(m).encode()

    nc.to_json_bytes = patched


def _host_inputs(x, W, b, part_idx):
    """Build the 8 per-core input maps."""
    import ml_dtypes

    mm_np = ml_dtypes.bfloat16 if MAIN_BF16 else np.float32
    xT = x.T.reshape(KT, 128, B).astype(mm_np)                # [KT,128,B]
    in_maps = []
    for i in range(NCORES):
        sl = slice(i * PL, (i + 1) * PL)
        WT = W[sl].transpose(2, 0, 1).reshape(
            KT, 128, PL * Q).astype(mm_np)                    # [KT,128,PL*Q]
        xw = np.empty((KT, 128, B + PL * Q), dtype=mm_np)
        xw[:, :, :B] = xT
        xw[:, :, B:] = WT
        bo = np.empty((1, PL * Q + B), dtype=mm_np)
        bo[0, :PL * Q] = b[sl].reshape(-1)
        bo[0, PL * Q:] = 1.0
        idxq = np.empty((Q, PL, C + 1), dtype=np.float16)
        idxq[:, :, :C] = part_idx[sl].astype(np.float16)[None, :, :]
        idxq[:, :, C] = np.arange(Q, dtype=np.float16)[:, None]
        in_maps.append({"xw": xw, "bo": bo, "idxq": idxq})
    return in_maps


def kernel(x, W, b, part_idx, _trace=False):
    from concourse.bass_utils import run_bass_kernel_spmd

    x = np.asarray(x, dtype=np.float32)
    W = np.asarray(W, dtype=np.float32)
    b = np.asarray(b, dtype=np.float32)
    part_idx = np.asarray(part_idx)

    nc = _build_nc()
    in_maps = _host_inputs(x, W, b, part_idx)
    res = run_bass_kernel_spmd(nc, in_maps, list(range(NCORES)),
                               trace=_trace)
    out = np.concatenate([r["out"] for r in res.results], axis=1)
    if _trace:
        return out, res
    return out



# revision 2
# speedup vs baseline: 1.0546x; 1.0546x over previous
"""Trainium2 Bass kernel v2.1 for nn_CombinatorialClassifier.

Computation (reference):
    logits = einsum('bf,pqf->bpq', x, W) + b        # [B,P,Q]
    logp   = log_softmax(logits, axis=2)            # [B,P,Q]
    out    = take_along_axis(logp, part_idx, 2)     # [B,P,C]

Shapes: B=256, P=64, Q=128, C=1000, F=2048.  Expert-parallel over P
(8 partitionings per core), full x on every core, no collectives.

Design:
  - main matmul in orientation B: psum_lin[b, (p,q)] += x_k.T @ W_k
    with x stationary and fp8e4 DoubleRow (2 contraction rows per
    partition, half the HBM bytes, 2x PE rate).  W scaled by 32 on the
    host; the 1/32 descale rides the ACT copies.  Mains are blk-major
    so blk0's softmax overlaps blk1's (SBUF-resident) mains.
  - softmax: wide ACT exps (one per psum-lin tile) + DVE tensor_reduce
    for sumexp in the [b, 1] orientation; Ln / negate on ACT.  lse is
    folded into the og drain copies per-partition, so the gather
    operates on raw rescaled logits.
  - gather is a one-hot matmul; the one-hot is host-built fp8e4 (exact
    0/1).  linsc is transposed to [q, b] via PE transposes (identity
    DMA'd once) into per-blk psum tiles.
  - the tail drain (psum_T -> logpT, psum_out -> og) is BLOCK
    SPECIALIZED: blk0's chain runs entirely on DVE, blk1's entirely on
    ACT, so the two chains overlap and every gather matmul needs only
    one semaphore wait (this walrus build encodes at most one sync
    wait per instruction).  Output DMAs are issued by the chain's own
    engine so they inherit its clock and carry only the queue wait.
  - og/output are bf16 (host upcasts); each og op drains both 500-col
    psum chunks of a p in one 3D-AP instruction.
  - 3 PE dummy matmuls at the blk boundary absorb blk0's psum-slot
    WARs so blk1's gathers stay single-wait.
"""

import numpy as np

B, P, Q, C, F = 256, 64, 128, 1000, 2048
NCORES = 8
PL = P // NCORES          # partitionings per core
KT2 = 8                   # K tiles of 256 (128 partitions x DoubleRow 2)
XC = B                    # x columns in the xw stream
WC = PL * Q               # W columns in the xw stream
NBLK = B // 128           # b blocks
WSCALE = 32.0


def _build_nc():
    import concourse.bass as bass
    import concourse.tile as tile
    from concourse import mybir
    from contextlib import ExitStack

    DT = mybir.dt.float32
    BF = mybir.dt.bfloat16
    F8 = mybir.dt.float8e4
    ACT = mybir.ActivationFunctionType

    nc = bass.Bass()
    xw_d = nc.declare_dram_parameter("xw", [KT2, 128, 2, XC + WC], F8,
                                     isOutput=False)
    bo_d = nc.declare_dram_parameter("bo", [1, WC + 128], BF, isOutput=False)
    id_d = nc.declare_dram_parameter("ident", [128, 128], BF, isOutput=False)
    oh_d = nc.declare_dram_parameter("oh", [128, PL * C], F8, isOutput=False)
    out_d = nc.declare_dram_parameter("out", [B, PL, C], BF, isOutput=True)

    with ExitStack() as ctx:
        tc = ctx.enter_context(tile.TileContext(nc))
        singles = ctx.enter_context(tc.tile_pool(name="singles", bufs=1))
        ps_t_ctx = ExitStack()
        ps_t = ps_t_ctx.enter_context(
            tc.tile_pool(name="ps_t", bufs=1, space=bass.MemorySpace.PSUM))
        ps_t_close = ps_t_ctx.close
        lin_ctx = ExitStack()
        ps_lin = lin_ctx.enter_context(
            tc.tile_pool(name="ps_lin", bufs=1, space=bass.MemorySpace.PSUM))

        def fresh(shape, dtype, tag):
            return singles.tile(shape, dtype, tag=tag, name=tag)

        # ---- input DMAs, all on the SP HWDGE queue (ordered sems) ---
        bo_sb = fresh([1, WC + 128], BF, "bo")
        nc.sync.dma_start(out=bo_sb[:], in_=bo_d[:])
        id_sb = fresh([128, 128], BF, "ident")
        nc.sync.dma_start(out=id_sb[:], in_=id_d[:])
        xwk = []
        for k in range(KT2):
            t = fresh([128, 2, XC + WC], F8, f"xwk{k}")
            nc.sync.dma_start(out=t[:], in_=xw_d[k])
            xwk.append(t)
        oh_sb = fresh([128, PL * C], F8, "oh")
        nc.sync.dma_start(out=oh_sb[:], in_=oh_d[:])

        # ---- PE: dummy transpose consumes ident's DMA sem early -----
        pst = {}
        for blk in range(NBLK):
            pst[blk] = ps_t.tile([128, PL, 128], BF, name=f"pst{blk}")
        nc.tensor.transpose(pst[0][:, 0, :], id_sb[:], id_sb[:])

        # ---- PE: bias openers + blk-major DoubleRow mains ------------
        lin = {}
        for blk in range(NBLK):
            for h in range(2):
                t = ps_lin.tile([128, 4, 128], DT, name=f'lin{blk}_{h}')
                lin[(blk, h)] = t
                nc.tensor.matmul(
                    t[:, :, :],
                    bo_sb[0:1, WC:WC + 128],
                    bo_sb[0:1, h * 512:(h + 1) * 512],
                    start=True, stop=False)
        for blk in range(NBLK):
            for k in range(KT2):
                for h in range(2):
                    nc.tensor.matmul(
                        lin[(blk, h)][:, :, :],
                        xwk[k][:, :, blk * 128:(blk + 1) * 128],
                        xwk[k][:, :, XC + h * 512:XC + (h + 1) * 512],
                        start=False, stop=(k == KT2 - 1),
                        perf_mode=mybir.MatmulPerfMode.DoubleRow)

        # PE observer: absorb oh's DMA sem so gathers carry only their
        # drain-chain wait
        nc.tensor.ldweights(oh_sb[:, 0:1])

        # ---- softmax prologue per blk --------------------------------
        obs_junk = fresh([1, 8], DT, "obs_junk")
        warm_junk = fresh([1, 2], DT, "warm_junk")
        nc.scalar.activation(out=warm_junk[0:1, 0:1],
                             in_=warm_junk[0:1, 1:2],
                             func=ACT.Identity, scale=0.0, bias=0.0)

        linsc, lse, neg_lse, exps, sumexp = {}, {}, {}, {}, {}

        def act_linsc(blk):
            for h in range(2):
                t = fresh([128, 4, 128], BF, f"linsc{blk}_{h}")
                linsc[(blk, h)] = t
                nc.scalar.activation(out=t[:, :, :], in_=lin[(blk, h)][:, :, :],
                                     func=ACT.Copy, scale=1.0 / WSCALE,
                                     bias=0.0)

        def act_exp(blk):
            for h in range(2):
                e = fresh([128, 4, 128], BF, f"exp{blk}_{h}")
                exps[(blk, h)] = e
                nc.scalar.activation(out=e[:, :, :], in_=lin[(blk, h)][:, :, :],
                                     func=ACT.Exp, scale=1.0 / WSCALE)

        def dve_red(blk):
            sumexp[blk] = fresh([128, PL], DT, f"sumexp{blk}")
            for h in range(2):
                nc.vector.tensor_reduce(
                    out=sumexp[blk][:, h * 4:(h + 1) * 4],
                    in_=exps[(blk, h)][:, :, :],
                    axis=mybir.AxisListType.X, op=mybir.AluOpType.add)

        def act_post(blk):
            t = fresh([128, PL], DT, f"lse{blk}")
            lse[blk] = t
            nc.scalar.activation(out=t[:], in_=sumexp[blk][:], func=ACT.Ln)
            t2 = fresh([128, PL], DT, f"neglse{blk}")
            neg_lse[blk] = t2
            nc.scalar.activation(out=t2[:], in_=lse[blk][:],
                                 func=ACT.Identity, scale=-1.0)
            # ACT self-absorber for the neg_lse RAW
            aabs = fresh([1, 1], DT, f"aabs{blk}")
            nc.scalar.activation(out=aabs[:], in_=neg_lse[blk][0:1, 0:1],
                                 func=ACT.Copy, bias=0.0, scale=1.0)

        act_linsc(0)
        act_linsc(1)
        act_exp(0)
        act_exp(1)
        dve_red(0)
        act_post(0)
        # DVE absorber: pull ACT@neg_lse0 into DVE's clock
        dabs = fresh([128, 1], DT, "dabs0")
        nc.vector.tensor_copy(dabs[:], neg_lse[0][:, 0:1])

        # ---- transposes + logpT copies (blk0 -> DVE, blk1 -> ACT) ----
        logpT = {}
        for blk in range(NBLK):
            for p in range(PL):
                nc.tensor.transpose(pst[blk][:, p, :],
                                    linsc[(blk, p // 4)][:, p % 4, :],
                                    id_sb[:])
        for p in range(PL):
            t = fresh([128, 128], BF, f"logpT0_{p}")
            logpT[(0, p)] = t
            nc.vector.tensor_copy(t[:], pst[0][:, p, :])
        dve_red(1)
        act_post(1)
        for p in range(PL):
            t = fresh([128, 128], BF, f"logpT1_{p}")
            logpT[(1, p)] = t
            nc.scalar.activation(out=t[:], in_=pst[1][:, p, :],
                                 func=ACT.Copy, bias=0.0, scale=1.0)

        # lin + pst banks free; gather slots reuse them (bufs=4 so the
        # psum-slot WAR of each gather lands on its own chain: 4 slots
        # back = 2 p's back = same blk in the interleaved stream)
        lin_ctx.close()
        ps_t_close()
        ps_out = ctx.enter_context(
            tc.tile_pool(name="ps_out", bufs=4, space=bass.MemorySpace.PSUM))

        # PE LDW observers: absorb both drain chains' copy sems so the
        # first-rotation gathers' bank-reuse WARs are covered
        nc.tensor.ldweights(logpT[(0, PL - 1)][:, 0:1])
        nc.tensor.ldweights(logpT[(1, PL - 1)][:, 0:1])

        og_tiles = {}
        og_last = {}

        def gather_p(blk, p, drain):
            po = ps_out.tile([128, 2, 512], DT, name='po')
            for ci in range(2):
                nc.tensor.matmul(
                    po[:, ci, 0:500], logpT[(blk, p)][:],
                    oh_sb[:, p * C + ci * 500:p * C + ci * 500 + 500],
                    start=True, stop=True)
            pair = p // 2
            if p % 2 == 0:
                og_tiles[(blk, pair)] = fresh([128, 2, 2, 500], BF,
                                              f"og{blk}_{pair}")
            og = og_tiles[(blk, pair)]
            if drain == 'dve':
                og_last[(blk, pair)] = nc.vector.tensor_scalar(
                    out=og[:, p % 2, :, :], in0=po[:, :, 0:500],
                    scalar1=lse[blk][:, p:p + 1], scalar2=None,
                    op0=mybir.AluOpType.subtract)
            else:
                og_last[(blk, pair)] = nc.scalar.activation(
                    out=og[:, p % 2, :, :], in_=po[:, :, 0:500],
                    func=ACT.Identity, scale=1.0,
                    bias=neg_lse[blk][:, p:p + 1])
            if p % 2 == 1:
                bsl = slice(blk * 128, (blk + 1) * 128)
                # Pool observer (og is SBUF; Pool may touch SBUF) absorbs
                # the drain engine's sem, then the SWDGE DMA inherits it
                obs = nc.gpsimd.tensor_copy(
                    obs_junk[0:1, blk * 4 + pair:blk * 4 + pair + 1],
                    og[0:1, 1, 1, 499:500])
                dma = nc.gpsimd.dma_start(
                    out=out_d[bsl, pair * 2:pair * 2 + 2, :], in_=og[:])
                tile.add_dep_helper(dma.ins, obs.ins, sync=False,
                                    reason="dma after pool obs")

        # ---- per-p interleaved gathers: both drain chains run hot ----
        for p in range(PL):
            gather_p(0, p, 'dve')
            gather_p(1, p, 'act')

    _install_drain_split(nc)
    return nc


def _install_drain_split(nc, chunk=1):
    """Legalize sync for this walrus build (at most ONE sync wait per
    instruction):

    1. Vector-clock pass: compute, for every instruction, the set of
       instructions provably COMPLETED before it dispatches — via its
       own sem waits (a wait S>=v proves every update contributing to
       values 1..v completed, and transitively everything those
       instructions' dispatch-clocks contain) plus same-engine dispatch
       order (an engine dispatches in program order, so anything done
       before a predecessor's dispatch is done before ours).  Any
       emitted wait already implied by the rest is dropped.
    2. Remaining multi-wait Drains are split into single-wait chains.
    """
    import copy
    import json

    orig = nc.to_json_bytes

    def patched():
        m = json.loads(orig())
        for fn in m["functions"]:
            insts = []
            for bb in fn["blocks"]:
                insts.extend(bb["instructions"])
            n = len(insts)
            # sem name -> list of (cum_value, idx) in completion order
            updates = {}
            cum = {}
            for i, inst in enumerate(insts):
                si = inst.get("sync_info") or {}
                for u in (si.get("on_update") or []):
                    s = u["ant_name"]
                    cum[s] = cum.get(s, 0) + u.get("update_value", 1)
                    updates.setdefault(s, []).append((cum[s], i))

            def targets(s, v):
                """instruction idxs whose updates are needed for sem s
                to reach v"""
                return [i for (c, i) in updates.get(s, []) if c <= v]

            eng_pred = {}
            last = {}
            for i, inst in enumerate(insts):
                e = inst.get("engine", "?")
                eng_pred[i] = last.get(e)
                last[e] = i

            done = [set() for _ in range(n)]
            for _ in range(64):
                changed = False
                for i, inst in enumerate(insts):
                    d = set()
                    if eng_pred[i] is not None:
                        p = eng_pred[i]
                        d |= done[p]
                    si = inst.get("sync_info") or {}
                    for w in (si.get("on_wait") or []):
                        for j in targets(w["ant_name"], w["wait_value"]):
                            d.add(j)
                            d |= done[j]
                    if d != done[i]:
                        done[i] = d
                        changed = True
                if not changed:
                    break

            # drop implied waits on multi-wait instructions
            for i, inst in enumerate(insts):
                si = inst.get("sync_info") or {}
                waits = si.get("on_wait") or []
                if len(waits) <= 1:
                    continue
                base = set()
                if eng_pred[i] is not None:
                    base |= done[eng_pred[i]]
                keep = list(waits)
                for w in list(keep):
                    others = set(base)
                    for w2 in keep:
                        if w2 is w:
                            continue
                        for j in targets(w2["ant_name"], w2["wait_value"]):
                            others.add(j)
                            others |= done[j]
                    if all(j in others
                           for j in targets(w["ant_name"], w["wait_value"])):
                        keep.remove(w)
                        if len(keep) <= 1:
                            break
                si["on_wait"] = keep

        # split any remaining multi-wait Drains
        for fn in m["functions"]:
            for bb in fn["blocks"]:
                out = []
                for inst in bb["instructions"]:
                    si = inst.get("sync_info")
                    if (si and si.get("on_wait")
                            and len(si["on_wait"]) > chunk):
                        if inst.get("opcode") != "Drain":
                            raise RuntimeError(
                                f"multi-wait survives legalization: "
                                f"{inst.get('opcode')} {inst.get('name')} "
                                f"{si['on_wait']}")
                        waits = si["on_wait"]
                        head, keep = waits[:-chunk], waits[-chunk:]
                        for j in range(0, len(head), chunk):
                            clone = copy.deepcopy(inst)
                            clone["name"] = f"{inst['name']}-ds{j}"
                            clone["sync_info"] = {
                                "on_wait": head[j:j + chunk],
                                "on_update": [],
                            }
                            out.append(clone)
                        si["on_wait"] = keep
                    out.append(inst)
                bb["instructions"] = out
        return json.dumps(m).encode()

    nc.to_json_bytes = patched


def _host_inputs(x, W, b, part_idx):
    import ml_dtypes
    f8 = ml_dtypes.float8_e4m3
    bf = ml_dtypes.bfloat16

    # x: [B, F] -> [KT2, 128, 2, B] with f = 256*k2 + 128*j + r
    xT = np.ascontiguousarray(
        x.T.reshape(KT2, 2, 128, B).transpose(0, 2, 1, 3)).astype(f8)
    ident = np.eye(128, dtype=np.float32).astype(bf)
    qarange = np.arange(Q, dtype=np.int64)

    in_maps = []
    for i in range(NCORES):
        sl = slice(i * PL, (i + 1) * PL)
        Wt = (W[sl] * WSCALE).transpose(2, 0, 1).reshape(
            KT2, 2, 128, PL * Q).transpose(0, 2, 1, 3)
        xw = np.empty((KT2, 128, 2, XC + WC), dtype=f8)
        xw[:, :, :, :XC] = xT
        xw[:, :, :, XC:] = Wt.astype(f8)
        bo = np.empty((1, WC + 128), dtype=bf)
        bo[0, :WC] = (b[sl].reshape(-1) * WSCALE).astype(bf)
        bo[0, WC:] = 1.0
        oh = (qarange[:, None, None] == part_idx[sl][None, :, :]
              ).reshape(128, PL * C).astype(f8)
        in_maps.append({"xw": xw, "bo": bo, "ident": ident, "oh": oh})
    return in_maps


def kernel(x, W, b, part_idx, _trace=False):
    from concourse.bass_utils import run_bass_kernel_spmd

    x = np.asarray(x, dtype=np.float32)
    W = np.asarray(W, dtype=np.float32)
    b = np.asarray(b, dtype=np.float32)
    part_idx = np.asarray(part_idx)

    nc = _build_nc()
    in_maps = _host_inputs(x, W, b, part_idx)
    res = run_bass_kernel_spmd(nc, in_maps, list(range(NCORES)),
                               trace=_trace)
    out = np.concatenate(
        [np.asarray(r["out"], dtype=np.float32) for r in res.results], axis=1)
    if _trace:
        return out, res
    return out


# revision 3
# speedup vs baseline: 1.0722x; 1.0167x over previous
"""Trainium2 Bass kernel v2.1 for nn_CombinatorialClassifier.

Computation (reference):
    logits = einsum('bf,pqf->bpq', x, W) + b        # [B,P,Q]
    logp   = log_softmax(logits, axis=2)            # [B,P,Q]
    out    = take_along_axis(logp, part_idx, 2)     # [B,P,C]

Shapes: B=256, P=64, Q=128, C=1000, F=2048.  Expert-parallel over P
(8 partitionings per core), full x on every core, no collectives.

Design:
  - main matmul in orientation B: psum_lin[b, (p,q)] += x_k.T @ W_k
    with x stationary and fp8e4 DoubleRow (2 contraction rows per
    partition, half the HBM bytes, 2x PE rate).  W scaled by 32 on the
    host; the 1/32 descale rides the ACT copies.  Mains are blk-major
    so blk0's softmax overlaps blk1's (SBUF-resident) mains.
  - softmax: wide ACT exps (one per psum-lin tile) + DVE tensor_reduce
    for sumexp in the [b, 1] orientation; Ln / negate on ACT.  lse is
    folded into the og drain copies per-partition, so the gather
    operates on raw rescaled logits.
  - gather is a one-hot matmul; the one-hot is host-built fp8e4 (exact
    0/1).  linsc is transposed to [q, b] via PE transposes (identity
    DMA'd once) into per-blk psum tiles.
  - the tail drain (psum_T -> logpT, psum_out -> og) is BLOCK
    SPECIALIZED: blk0's chain runs entirely on DVE, blk1's entirely on
    ACT, so the two chains overlap and every gather matmul needs only
    one semaphore wait (this walrus build encodes at most one sync
    wait per instruction).  Output DMAs are issued by the chain's own
    engine so they inherit its clock and carry only the queue wait.
  - og/output are bf16 (host upcasts); each og op drains both 500-col
    psum chunks of a p in one 3D-AP instruction.
  - 3 PE dummy matmuls at the blk boundary absorb blk0's psum-slot
    WARs so blk1's gathers stay single-wait.
"""

import numpy as np

B, P, Q, C, F = 256, 64, 128, 1000, 2048
NCORES = 8
PL = P // NCORES          # partitionings per core
KT2 = 8                   # K tiles of 256 (128 partitions x DoubleRow 2)
XC = B                    # x columns in the xw stream
WC = PL * Q               # W columns in the xw stream
NBLK = B // 128           # b blocks
WSCALE = 32.0


def _build_nc():
    import concourse.bass as bass
    import concourse.tile as tile
    from concourse import mybir
    from contextlib import ExitStack

    DT = mybir.dt.float32
    BF = mybir.dt.bfloat16
    F8 = mybir.dt.float8e4
    ACT = mybir.ActivationFunctionType

    nc = bass.Bass()
    xw_d = nc.declare_dram_parameter("xw", [KT2, 128, 2, XC + WC], F8,
                                     isOutput=False)
    bo_d = nc.declare_dram_parameter("bo", [1, WC + 128], BF, isOutput=False)
    id_d = nc.declare_dram_parameter("ident", [128, 128], BF, isOutput=False)
    oh_d = nc.declare_dram_parameter("oh", [128, PL * C], F8, isOutput=False)
    out_d = nc.declare_dram_parameter("out", [B, PL, C], BF, isOutput=True)

    with ExitStack() as ctx:
        tc = ctx.enter_context(tile.TileContext(nc))
        singles = ctx.enter_context(tc.tile_pool(name="singles", bufs=1))
        ps_t_ctx = ExitStack()
        ps_t = ps_t_ctx.enter_context(
            tc.tile_pool(name="ps_t", bufs=1, space=bass.MemorySpace.PSUM))
        ps_t_close = ps_t_ctx.close
        lin_ctx = ExitStack()
        ps_lin = lin_ctx.enter_context(
            tc.tile_pool(name="ps_lin", bufs=1, space=bass.MemorySpace.PSUM))

        def fresh(shape, dtype, tag):
            return singles.tile(shape, dtype, tag=tag, name=tag)

        # ---- input DMAs, all on the SP HWDGE queue (ordered sems) ---
        bo_sb = fresh([1, WC + 128], BF, "bo")
        nc.sync.dma_start(out=bo_sb[:], in_=bo_d[:])
        id_sb = fresh([128, 128], BF, "ident")
        nc.sync.dma_start(out=id_sb[:], in_=id_d[:])
        xwk = []
        for k in range(KT2):
            t = fresh([128, 2, XC + WC], F8, f"xwk{k}")
            nc.sync.dma_start(out=t[:], in_=xw_d[k])
            xwk.append(t)
        oh_sb = fresh([128, PL * C], F8, "oh")
        nc.sync.dma_start(out=oh_sb[:], in_=oh_d[:])

        # ---- PE: dummy transpose consumes ident's DMA sem early -----
        pst = {}
        for blk in range(NBLK):
            pst[blk] = ps_t.tile([128, PL, 128], BF, name=f"pst{blk}")
        nc.tensor.transpose(pst[0][:, 0, :], id_sb[:], id_sb[:])

        # ---- PE: bias openers + blk-major DoubleRow mains ------------
        lin = {}
        for blk in range(NBLK):
            for h in range(2):
                t = ps_lin.tile([128, 4, 128], DT, name=f'lin{blk}_{h}')
                lin[(blk, h)] = t
                nc.tensor.matmul(
                    t[:, :, :],
                    bo_sb[0:1, WC:WC + 128],
                    bo_sb[0:1, h * 512:(h + 1) * 512],
                    start=True, stop=False)
        for blk in range(NBLK):
            for k in range(KT2):
                for h in range(2):
                    nc.tensor.matmul(
                        lin[(blk, h)][:, :, :],
                        xwk[k][:, :, blk * 128:(blk + 1) * 128],
                        xwk[k][:, :, XC + h * 512:XC + (h + 1) * 512],
                        start=False, stop=(k == KT2 - 1),
                        perf_mode=mybir.MatmulPerfMode.DoubleRow)

        # PE observer: absorb oh's DMA sem so gathers carry only their
        # drain-chain wait
        nc.tensor.ldweights(oh_sb[:, 0:1])

        # ---- softmax prologue per blk --------------------------------
        obs_junk = fresh([1, 8], DT, "obs_junk")
        warm_junk = fresh([1, 2], DT, "warm_junk")
        nc.scalar.activation(out=warm_junk[0:1, 0:1],
                             in_=warm_junk[0:1, 1:2],
                             func=ACT.Identity, scale=0.0, bias=0.0)

        linsc, lse, neg_lse, exps, sumexp = {}, {}, {}, {}, {}

        def mk_linsc(blk, eng):
            for h in range(2):
                t = fresh([128, 4, 128], BF, f"linsc{blk}_{h}")
                linsc[(blk, h)] = t
                if eng == 'act':
                    nc.scalar.activation(out=t[:, :, :],
                                         in_=lin[(blk, h)][:, :, :],
                                         func=ACT.Copy, scale=1.0 / WSCALE,
                                         bias=0.0)
                else:
                    nc.vector.tensor_scalar_mul(t[:, :, :],
                                                lin[(blk, h)][:, :, :],
                                                1.0 / WSCALE)

        def act_exp(blk):
            for h in range(2):
                e = fresh([128, 4, 128], BF, f"exp{blk}_{h}")
                exps[(blk, h)] = e
                nc.scalar.activation(out=e[:, :, :], in_=lin[(blk, h)][:, :, :],
                                     func=ACT.Exp, scale=1.0 / WSCALE)

        def dve_red(blk):
            sumexp[blk] = fresh([128, PL], DT, f"sumexp{blk}")
            for h in range(2):
                nc.vector.tensor_reduce(
                    out=sumexp[blk][:, h * 4:(h + 1) * 4],
                    in_=exps[(blk, h)][:, :, :],
                    axis=mybir.AxisListType.X, op=mybir.AluOpType.add)

        def act_post(blk):
            t = fresh([128, PL], DT, f"lse{blk}")
            lse[blk] = t
            nc.scalar.activation(out=t[:], in_=sumexp[blk][:], func=ACT.Ln)
            t2 = fresh([128, PL], DT, f"neglse{blk}")
            neg_lse[blk] = t2
            nc.scalar.activation(out=t2[:], in_=lse[blk][:],
                                 func=ACT.Identity, scale=-1.0)
            # ACT self-absorber for the neg_lse RAW
            aabs = fresh([1, 1], DT, f"aabs{blk}")
            nc.scalar.activation(out=aabs[:], in_=neg_lse[blk][0:1, 0:1],
                                 func=ACT.Copy, bias=0.0, scale=1.0)

        act_exp(0)
        mk_linsc(0, 'dve')
        act_exp(1)
        mk_linsc(1, 'act')
        dve_red(0)
        act_post(0)

        # ---- transposes + logpT copies (blk0 -> DVE, blk1 -> ACT) ----
        logpT = {}
        for blk in range(NBLK):
            for p in range(PL):
                nc.tensor.transpose(pst[blk][:, p, :],
                                    linsc[(blk, p // 4)][:, p % 4, :],
                                    id_sb[:])
        for p in range(PL):
            t = fresh([128, 128], BF, f"logpT0_{p}")
            logpT[(0, p)] = t
            nc.vector.tensor_copy(t[:], pst[0][:, p, :])
        # DVE absorber: pull ACT@neg_lse0 into DVE's clock before og0
        dabs = fresh([128, 1], DT, "dabs0")
        nc.vector.tensor_copy(dabs[:], neg_lse[0][:, 0:1])
        dve_red(1)
        for p in range(PL):
            t = fresh([128, 128], BF, f"logpT1_{p}")
            logpT[(1, p)] = t
            nc.scalar.activation(out=t[:], in_=pst[1][:, p, :],
                                 func=ACT.Copy, bias=0.0, scale=1.0)
        act_post(1)

        # lin + pst banks free; gather slots reuse them (bufs=4 so the
        # psum-slot WAR of each gather lands on its own chain: 4 slots
        # back = 2 p's back = same blk in the interleaved stream)
        lin_ctx.close()
        ps_t_close()
        ps_out = ctx.enter_context(
            tc.tile_pool(name="ps_out", bufs=4, space=bass.MemorySpace.PSUM))

        # PE LDW observers: absorb both drain chains' copy sems so the
        # first-rotation gathers' bank-reuse WARs are covered
        nc.tensor.ldweights(logpT[(0, PL - 1)][:, 0:1])
        nc.tensor.ldweights(logpT[(1, PL - 1)][:, 0:1])

        og_tiles = {}
        og_last = {}

        def gather_p(blk, p, drain):
            po = ps_out.tile([128, 2, 512], DT, name='po')
            for ci in range(2):
                nc.tensor.matmul(
                    po[:, ci, 0:500], logpT[(blk, p)][:],
                    oh_sb[:, p * C + ci * 500:p * C + ci * 500 + 500],
                    start=True, stop=True)
            pair = p // 2
            if p % 2 == 0:
                og_tiles[(blk, pair)] = fresh([128, 2, 2, 500], BF,
                                              f"og{blk}_{pair}")
            og = og_tiles[(blk, pair)]
            if drain == 'dve':
                og_last[(blk, pair)] = nc.vector.tensor_scalar(
                    out=og[:, p % 2, :, :], in0=po[:, :, 0:500],
                    scalar1=lse[blk][:, p:p + 1], scalar2=None,
                    op0=mybir.AluOpType.subtract)
            else:
                og_last[(blk, pair)] = nc.scalar.activation(
                    out=og[:, p % 2, :, :], in_=po[:, :, 0:500],
                    func=ACT.Identity, scale=1.0,
                    bias=neg_lse[blk][:, p:p + 1])
            if p % 2 == 1:
                bsl = slice(blk * 128, (blk + 1) * 128)
                # HWDGE out-DMAs: SP-issued for the DVE chain (its data
                # wait is the single sem; the queue-pred wait is provably
                # satisfied -- the whole input queue is in the og chain's
                # past -- so the legalizer drops it), ACT-issued for the
                # ACT chain (inherits in-order).
                eng = nc.sync if drain == 'dve' else nc.scalar
                dma = eng.dma_start(
                    out=out_d[bsl, pair * 2:pair * 2 + 2, :], in_=og[:])
                tile.add_dep_helper(dma.ins, og_last[(blk, pair)].ins,
                                    sync=False, reason="dma after og")

        # ---- per-p interleaved gathers: both drain chains run hot ----
        for p in range(PL):
            gather_p(0, p, 'dve')
            gather_p(1, p, 'act')

    _install_drain_split(nc)
    return nc


def _install_drain_split(nc, chunk=1):
    """Legalize sync for this walrus build (at most ONE sync wait per
    instruction):

    1. Vector-clock pass: compute, for every instruction, the set of
       instructions provably COMPLETED before it dispatches — via its
       own sem waits (a wait S>=v proves every update contributing to
       values 1..v completed, and transitively everything those
       instructions' dispatch-clocks contain) plus same-engine dispatch
       order (an engine dispatches in program order, so anything done
       before a predecessor's dispatch is done before ours).  Any
       emitted wait already implied by the rest is dropped.
    2. Remaining multi-wait Drains are split into single-wait chains.
    """
    import copy
    import json

    orig = nc.to_json_bytes

    def patched():
        m = json.loads(orig())
        for fn in m["functions"]:
            insts = []
            for bb in fn["blocks"]:
                insts.extend(bb["instructions"])
            n = len(insts)
            # sem name -> list of (cum_value, idx) in completion order
            updates = {}
            cum = {}
            for i, inst in enumerate(insts):
                si = inst.get("sync_info") or {}
                for u in (si.get("on_update") or []):
                    s = u["ant_name"]
                    cum[s] = cum.get(s, 0) + u.get("update_value", 1)
                    updates.setdefault(s, []).append((cum[s], i))

            def targets(s, v):
                """instruction idxs whose updates are needed for sem s
                to reach v"""
                return [i for (c, i) in updates.get(s, []) if c <= v]

            eng_pred = {}
            last = {}
            for i, inst in enumerate(insts):
                e = inst.get("engine", "?")
                eng_pred[i] = last.get(e)
                last[e] = i

            done = [set() for _ in range(n)]
            for _ in range(64):
                changed = False
                for i, inst in enumerate(insts):
                    d = set()
                    if eng_pred[i] is not None:
                        p = eng_pred[i]
                        d |= done[p]
                    si = inst.get("sync_info") or {}
                    for w in (si.get("on_wait") or []):
                        for j in targets(w["ant_name"], w["wait_value"]):
                            d.add(j)
                            d |= done[j]
                    if d != done[i]:
                        done[i] = d
                        changed = True
                if not changed:
                    break

            # drop implied waits on multi-wait instructions
            for i, inst in enumerate(insts):
                si = inst.get("sync_info") or {}
                waits = si.get("on_wait") or []
                if len(waits) <= 1:
                    continue
                base = set()
                if eng_pred[i] is not None:
                    base |= done[eng_pred[i]]
                keep = list(waits)
                for w in list(keep):
                    others = set(base)
                    for w2 in keep:
                        if w2 is w:
                            continue
                        for j in targets(w2["ant_name"], w2["wait_value"]):
                            others.add(j)
                            others |= done[j]
                    if all(j in others
                           for j in targets(w["ant_name"], w["wait_value"])):
                        keep.remove(w)
                        if len(keep) <= 1:
                            break
                si["on_wait"] = keep

        # split any remaining multi-wait Drains
        for fn in m["functions"]:
            for bb in fn["blocks"]:
                out = []
                for inst in bb["instructions"]:
                    si = inst.get("sync_info")
                    if (si and si.get("on_wait")
                            and len(si["on_wait"]) > chunk):
                        if inst.get("opcode") != "Drain":
                            raise RuntimeError(
                                f"multi-wait survives legalization: "
                                f"{inst.get('opcode')} {inst.get('name')} "
                                f"{si['on_wait']}")
                        waits = si["on_wait"]
                        head, keep = waits[:-chunk], waits[-chunk:]
                        for j in range(0, len(head), chunk):
                            clone = copy.deepcopy(inst)
                            clone["name"] = f"{inst['name']}-ds{j}"
                            clone["sync_info"] = {
                                "on_wait": head[j:j + chunk],
                                "on_update": [],
                            }
                            out.append(clone)
                        si["on_wait"] = keep
                    out.append(inst)
                bb["instructions"] = out
        return json.dumps(m).encode()

    nc.to_json_bytes = patched


def _host_inputs(x, W, b, part_idx):
    import ml_dtypes
    f8 = ml_dtypes.float8_e4m3
    bf = ml_dtypes.bfloat16

    # x: [B, F] -> [KT2, 128, 2, B] with f = 256*k2 + 128*j + r
    xT = np.ascontiguousarray(
        x.T.reshape(KT2, 2, 128, B).transpose(0, 2, 1, 3)).astype(f8)
    ident = np.eye(128, dtype=np.float32).astype(bf)
    qarange = np.arange(Q, dtype=np.int64)

    in_maps = []
    for i in range(NCORES):
        sl = slice(i * PL, (i + 1) * PL)
        Wt = (W[sl] * WSCALE).transpose(2, 0, 1).reshape(
            KT2, 2, 128, PL * Q).transpose(0, 2, 1, 3)
        xw = np.empty((KT2, 128, 2, XC + WC), dtype=f8)
        xw[:, :, :, :XC] = xT
        xw[:, :, :, XC:] = Wt.astype(f8)
        bo = np.empty((1, WC + 128), dtype=bf)
        bo[0, :WC] = (b[sl].reshape(-1) * WSCALE).astype(bf)
        bo[0, WC:] = 1.0
        oh = (qarange[:, None, None] == part_idx[sl][None, :, :]
              ).reshape(128, PL * C).astype(f8)
        in_maps.append({"xw": xw, "bo": bo, "ident": ident, "oh": oh})
    return in_maps


def kernel(x, W, b, part_idx, _trace=False):
    from concourse.bass_utils import run_bass_kernel_spmd

    x = np.asarray(x, dtype=np.float32)
    W = np.asarray(W, dtype=np.float32)
    b = np.asarray(b, dtype=np.float32)
    part_idx = np.asarray(part_idx)

    nc = _build_nc()
    in_maps = _host_inputs(x, W, b, part_idx)
    res = run_bass_kernel_spmd(nc, in_maps, list(range(NCORES)),
                               trace=_trace)
    out = np.concatenate(
        [np.asarray(r["out"], dtype=np.float32) for r in res.results], axis=1)
    if _trace:
        return out, res
    return out
